# revision 1
# baseline (speedup 1.0000x reference)
"""Trainium2 Bass kernel for a transformer block with top-2-of-8 MoE FFN.

Strategy (8 NeuronCores, SPMD via run_bass_kernel_spmd):
  P1: attention block + LN2 + router logits. Token-sharded with balanced
      causal pairing (core c owns query blocks c and 15-c of each batch);
      every core computes LN1 + k/v for all tokens (uniform SPMD, no
      collectives), q/attention/proj only for its own 512 tokens.
      Activations kept feature-major [feature, token] so matmul chains need
      no transposes; bf16 matmuls, fp32 PSUM accumulation; router logits
      from fp32 h2 (keeps top-k routing faithful).
  Host: softmax + top-2 routing, per-expert gather of h2 columns.
  P2: expert-parallel MoE. Core e runs gelu(h2 @ fc_w[e]) @ pj_w[e] for
      tokens routed to expert e (capacity-padded), gate applied on device.
  Host: scatter-add expert outputs, exact fp32 residual correction
      (device residual base is bf16(x); host adds back x - bf16(x)).
"""

import math
import os
import numpy as np
import ml_dtypes

import concourse.bass as bass
import concourse.mybir as mybir
import concourse.tile as tile
from concourse import bacc
from concourse.bass_utils import run_bass_kernel_spmd

F32 = mybir.dt.float32
BF16 = mybir.dt.bfloat16
AX = mybir.AxisListType
ALU = mybir.AluOpType
ACTF = mybir.ActivationFunctionType

B, T, C, H, E, TOPK = 2, 2048, 1024, 16, 8, 2
HID = 4 * C
D = C // H            # 64
N = B * T             # 4096 tokens
P = 128
CP = C // P           # 8 c-tiles
HP = HID // P         # 32 hid-tiles
NB = T // P           # 16 query blocks per batch
NCORES = 8
OWN = 4 * P           # 512 own tokens per core (2 blocks x 2 batches)
CAP = 1280            # per-expert token capacity (max observed ~1071)
KCH = 512             # attention key-chunk (one psum bank)
INV_SQRT_D = 1.0 / math.sqrt(D)
GELU_C = math.sqrt(2.0 / math.pi)

BF = ml_dtypes.bfloat16


_tcnt = [0]


def _tl(pool, shape, dtype, tag, ncol=None, bufs=None):
    _tcnt[0] += 1
    kw = {} if bufs is None else {"bufs": bufs}
    t = pool.tile(shape, dtype, tag=tag, name=f"{tag}_{_tcnt[0]}", **kw)
    return t if ncol is None else t[:, :ncol]


def _part3(a, p=P):
    """[R, F] -> [p, R//p, F]; row r = o*p + pi maps to [pi, o, F]."""
    R, Fd = a.shape
    return np.ascontiguousarray(a.reshape(R // p, p, Fd).transpose(1, 0, 2))


def _part2(a, p=P):
    """[R] -> [p, R//p]; row r = o*p + pi."""
    a = np.asarray(a, np.float32)
    R = a.shape[0]
    return np.ascontiguousarray(a.reshape(R // p, p).T)


# ---------------------------------------------------------------------------
# P1: attention + LN2 + router logits
# ---------------------------------------------------------------------------

def build_p1():
    nc = bacc.Bacc(None, target_bir_lowering=False)

    xkv = nc.dram_tensor("xkv", (P, CP, N), BF16, kind="ExternalInput")
    xq = nc.dram_tensor("xq", (P, CP, OWN), F32, kind="ExternalInput")
    qpos = nc.dram_tensor("qpos", (P, 4), F32, kind="ExternalInput")
    kpos = nc.dram_tensor("kpos", (1, T), F32, kind="ExternalInput")
    wqk = nc.dram_tensor("wqk", (P, CP, 2 * C), BF16, kind="ExternalInput")
    wv = nc.dram_tensor("wv", (P, CP, C), BF16, kind="ExternalInput")
    bqk = nc.dram_tensor("bqk", (P, 2 * CP), F32, kind="ExternalInput")
    wproj = nc.dram_tensor("wproj", (P, CP, C), BF16, kind="ExternalInput")
    bc = nc.dram_tensor("bc", (P, CP), F32, kind="ExternalInput")
    ln1g = nc.dram_tensor("ln1g", (P, CP), F32, kind="ExternalInput")
    ln1b = nc.dram_tensor("ln1b", (P, CP), F32, kind="ExternalInput")
    ln2g = nc.dram_tensor("ln2g", (P, CP), F32, kind="ExternalInput")
    ln2b = nc.dram_tensor("ln2b", (P, CP), F32, kind="ExternalInput")
    wrout = nc.dram_tensor("wrout", (P, CP, E), F32, kind="ExternalInput")

    x2T = nc.dram_tensor("x2T", (P, CP, OWN), F32, kind="ExternalOutput")
    h2T = nc.dram_tensor("h2T", (P, CP, OWN), BF16, kind="ExternalOutput")
    logitsT = nc.dram_tensor("logitsT", (E, OWN), F32, kind="ExternalOutput")

    with tile.TileContext(nc) as tc:
        with (
            tc.tile_pool(name="const", bufs=1) as constp,
            tc.tile_pool(name="wpool", bufs=2) as wpool,
            tc.tile_pool(name="hch", bufs=2) as hchp,
            tc.tile_pool(name="kv", bufs=1) as kvp,
            tc.tile_pool(name="qown", bufs=1) as qownp,
            tc.tile_pool(name="yT", bufs=1) as yTp,
            tc.tile_pool(name="statrow", bufs=1) as statrp,
            tc.tile_pool(name="stat128", bufs=2) as statp,
            tc.tile_pool(name="att", bufs=2) as attp,
            tc.tile_pool(name="x2", bufs=1) as x2p,
            tc.tile_pool(name="tmp", bufs=2) as tmpp,
            tc.tile_pool(name="ps_big", bufs=3, space="PSUM") as psb,
            tc.tile_pool(name="ps_att", bufs=2, space="PSUM") as psa,
            tc.tile_pool(name="ps_y", bufs=1, space="PSUM") as psy,
            tc.tile_pool(name="ps_row", bufs=1, space="PSUM") as psr,
        ):
            # ---- constants ----
            ones_bf = constp.tile([P, 1], BF16)
            nc.any.memset(ones_bf[:], 1.0)
            ones_f = constp.tile([P, 1], F32)
            nc.any.memset(ones_f[:], 1.0)
            eps_sb = constp.tile([1, 1], F32)
            nc.any.memset(eps_sb[:], 1e-5)
            ident_f = constp.tile([P, P], F32)
            from concourse.masks import make_identity
            make_identity(nc, ident_f)
            qpos_sb = constp.tile([P, 4], F32)
            nc.sync.dma_start(qpos_sb[:], qpos[:])
            ln1g_sb = constp.tile([P, CP], F32); nc.sync.dma_start(ln1g_sb[:], ln1g[:])
            ln1b_sb = constp.tile([P, CP], F32); nc.sync.dma_start(ln1b_sb[:], ln1b[:])
            ln2g_sb = constp.tile([P, CP], F32); nc.sync.dma_start(ln2g_sb[:], ln2g[:])
            ln2b_sb = constp.tile([P, CP], F32); nc.sync.dma_start(ln2b_sb[:], ln2b[:])
            bqk_sb = constp.tile([P, 2 * CP], F32); nc.sync.dma_start(bqk_sb[:], bqk[:])
            bc_sb = constp.tile([P, CP], F32); nc.sync.dma_start(bc_sb[:], bc[:])
            wrout_sb = constp.tile([P, CP, E], F32); nc.sync.dma_start(wrout_sb[:], wrout[:])
            xq_sb = constp.tile([P, CP, OWN], F32); nc.sync.dma_start(xq_sb[:], xq[:])
            wv_sb = constp.tile([P, CP, C], BF16); nc.sync.dma_start(wv_sb[:], wv[:])

            # 0/1 causal validity masks (bf16) for the 4 own q-blocks
            masks = [_tl(constp, [P, T], BF16, f"mask{j}") for j in range(4)]
            for ch in range(T // 512):
                sl = slice(ch * 512, (ch + 1) * 512)
                kr = _tl(tmpp, [1, 512], F32, "lg", bufs=1)
                nc.sync.dma_start(kr[:], kpos[:, sl])
                kch = _tl(tmpp, [P, 512], F32, "kch", bufs=1)
                nc.gpsimd.partition_broadcast(kch[:], kr[:])
                for j in range(4):
                    nc.vector.tensor_scalar(
                        masks[j][:, sl], kch[:], qpos_sb[:, j:j + 1], None,
                        ALU.is_le)

            def ln_chunk(src, ncol, g_sb, b_sb, ones, out_f32, out_bf16,
                         ct_consumer=None):
                """LayerNorm over the partition (C) dim of feature-major
                src [P, CP, ncol] (SBUF tile AP, fp32 or bf16, dtype matching
                `ones`). Writes result to out_f32 and/or out_bf16 (either may
                be None); optionally calls ct_consumer(ct, h_f32_chunk)."""
                ps_mu = _tl(psr, [1, 512], F32, "ps_row", ncol)
                for ct in range(CP):
                    nc.tensor.matmul(ps_mu[:], ones[:], src[:, ct, :],
                                     start=(ct == 0), stop=(ct == CP - 1))
                mu = _tl(statrp, [1, 512], F32, "mu", ncol)
                nc.scalar.activation(mu[:], ps_mu[:], ACTF.Identity, scale=1.0 / C)
                ps_m2 = _tl(psr, [1, 512], F32, "ps_row", ncol)
                for ct in range(CP):
                    xsq = _tl(tmpp, [P, 512], src.dtype, "t512", ncol)
                    nc.vector.tensor_mul(xsq[:], src[:, ct, :], src[:, ct, :])
                    nc.tensor.matmul(ps_m2[:], ones[:], xsq[:],
                                     start=(ct == 0), stop=(ct == CP - 1))
                m2 = _tl(statrp, [1, 512], F32, "m2", ncol)
                nc.scalar.activation(m2[:], ps_m2[:], ACTF.Identity, scale=1.0 / C)
                musq = _tl(statrp, [1, 512], F32, "musq", ncol)
                nc.vector.tensor_mul(musq[:], mu[:], mu[:])
                nc.vector.tensor_sub(m2[:], m2[:], musq[:])
                nc.scalar.activation(musq[:], m2[:], ACTF.Sqrt, bias=eps_sb[:, 0:1])
                rstd = musq
                nc.vector.reciprocal(rstd[:], rstd[:])
                mu128 = _tl(statp, [P, 512], F32, "mu128", ncol)
                nc.gpsimd.partition_broadcast(mu128[:], mu[:])
                rstd128 = _tl(statp, [P, 512], F32, "rstd128", ncol)
                nc.gpsimd.partition_broadcast(rstd128[:], rstd[:])
                for ct in range(CP):
                    xc = _tl(tmpp, [P, 512], F32, "t512", ncol)
                    nc.vector.tensor_sub(xc[:], src[:, ct, :], mu128[:])
                    nc.vector.tensor_mul(xc[:], xc[:], rstd128[:])
                    if out_f32 is not None or ct_consumer is not None:
                        hct = (out_f32[:, ct, :] if out_f32 is not None
                               else _tl(tmpp, [P, 512], F32, "t512", ncol))
                        nc.vector.tensor_scalar(
                            hct[:], xc[:], g_sb[:, ct:ct + 1], b_sb[:, ct:ct + 1],
                            ALU.mult, ALU.add)
                        if out_bf16 is not None:
                            nc.vector.tensor_copy(out_bf16[:, ct, :], hct[:])
                        if ct_consumer is not None:
                            ct_consumer(ct, hct)
                    else:
                        nc.vector.tensor_scalar(
                            out_bf16[:, ct, :], xc[:], g_sb[:, ct:ct + 1],
                            b_sb[:, ct:ct + 1], ALU.mult, ALU.add)

            # ---- own q (all 4 blocks): LN1 + q matmul ----
            hq_sb = _tl(hchp, [P, CP, 512], BF16, "hch")
            ln_chunk(xq_sb, OWN, ln1g_sb, ln1b_sb, ones_f, None, hq_sb)
            qT_sb = _tl(qownp, [P, CP, OWN], BF16, "qT")
            for rt in range(CP):
                wt = _tl(wpool, [P, CP, P], BF16, "w_t")
                nc.sync.dma_start(wt[:], wqk[:, :, rt * P:(rt + 1) * P])
                ps = _tl(psb, [P, 512], F32, "ps_mm")
                for ct in range(CP):
                    nc.tensor.matmul(ps[:], wt[:, ct, :], hq_sb[:, ct, :],
                                     start=(ct == 0), stop=(ct == CP - 1))
                nc.scalar.activation(qT_sb[:, rt, :], ps[:], ACTF.Identity,
                                     bias=bqk_sb[:, rt:rt + 1])

            yT_sb = yTp.tile([P, CP, OWN], BF16)

            for b in range(B):
                # ---- streamed LN1 + k/v for this batch ----
                kT_b = _tl(kvp, [P, CP, T], BF16, "kT")
                v_b = _tl(kvp, [P, NB, C], BF16, "v")
                for ch in range(T // 512):
                    csl = slice(b * T + ch * 512, b * T + (ch + 1) * 512)
                    hch = _tl(hchp, [P, CP, 512], BF16, "hch")
                    nc.sync.dma_start(hch[:], xkv[:, :, csl])
                    ln_chunk(hch, 512, ln1g_sb, ln1b_sb, ones_bf, None, hch)
                    for rt in range(CP):
                        wt = _tl(wpool, [P, CP, P], BF16, "w_t")
                        nc.sync.dma_start(wt[:], wqk[:, :, C + rt * P:C + (rt + 1) * P])
                        ps = _tl(psb, [P, 512], F32, "ps_mm")
                        for ct in range(CP):
                            nc.tensor.matmul(ps[:], wt[:, ct, :], hch[:, ct, :],
                                             start=(ct == 0), stop=(ct == CP - 1))
                        nc.scalar.activation(kT_b[:, rt, ch * 512:(ch + 1) * 512],
                                             ps[:], ACTF.Identity,
                                             bias=bqk_sb[:, CP + rt:CP + rt + 1])
                    for tt in range(4):
                        gtt = ch * 4 + tt
                        for vc in range(C // 512):
                            vsl = slice(vc * 512, (vc + 1) * 512)
                            ps = _tl(psb, [P, 512], F32, "ps_mm")
                            for ct in range(CP):
                                nc.tensor.matmul(
                                    ps[:], hch[:, ct, tt * P:(tt + 1) * P],
                                    wv_sb[:, ct, vsl],
                                    start=(ct == 0), stop=(ct == CP - 1))
                            nc.vector.tensor_copy(v_b[:, gtt, vsl], ps[:])

                # ---- attention: 2 own q-blocks x 16 heads ----
                for jj in range(2 if not os.environ.get("P1_SKIP_ATTN") else 0):
                    slot = 2 * b + jj
                    mask = masks[slot]
                    qsl = slice(slot * P, (slot + 1) * P)
                    for h in range(H):
                        hr = (h * D) // P
                        ho = (h * D) % P
                        att_t = _tl(attp, [P, T], BF16, "att")
                        for kc in range(T // KCH):
                            ksl = slice(kc * KCH, (kc + 1) * KCH)
                            ps_s = _tl(psa, [P, KCH], F32, "ps_s")
                            nc.tensor.matmul(ps_s[:], qT_sb[ho:ho + D, hr, qsl],
                                             kT_b[ho:ho + D, hr, ksl],
                                             start=True, stop=True)
                            nc.scalar.activation(att_t[:, ksl], ps_s[:],
                                                 ACTF.Exp, scale=INV_SQRT_D)
                        # causal mask on the otherwise-idle gpsimd engine
                        nc.gpsimd.tensor_mul(att_t[:], att_t[:], mask[:])
                        attT = _tl(attp, [P, NB, P], BF16, "attT")
                        for kb in range(NB):
                            nc.sync.dma_start(attT[:, kb, :],
                                              att_t[:, kb * P:(kb + 1) * P],
                                              transpose=True)
                        rowsum = _tl(tmpp, [P, 1], F32, "rowsum")
                        nc.vector.reduce_sum(rowsum[:], att_t[:], axis=AX.X)
                        ps_rs = _tl(psy, [1, P], F32, "ps_rs")
                        nc.tensor.transpose(ps_rs[:], rowsum[:], ident_f[:])
                        ps_y = _tl(psy, [D, P], F32, "ps_y")
                        for kb in range(NB):
                            nc.tensor.matmul(ps_y[:], v_b[:, kb, h * D:(h + 1) * D],
                                             attT[:, kb, :],
                                             start=(kb == 0), stop=(kb == NB - 1))
                        rinv_row = _tl(tmpp, [1, P], F32, "rinvr", bufs=1)
                        nc.vector.reciprocal(rinv_row[:], ps_rs[:])
                        rinv64 = _tl(tmpp, [D, P], F32, "rinv64")
                        nc.gpsimd.partition_broadcast(rinv64[:], rinv_row[:])
                        if ho == 0:
                            nc.vector.tensor_mul(yT_sb[0:D, hr, qsl], ps_y[:], rinv64[:])
                        else:
                            # cross-partition move: scale+cast on DVE
                            # (lane-aligned), then shift partitions via DMA
                            ytmp = _tl(tmpp, [D, P], BF16, "ytmp", bufs=1)
                            nc.vector.tensor_mul(ytmp[:], ps_y[:], rinv64[:])
                            nc.sync.dma_start(yT_sb[ho:ho + D, hr, qsl], ytmp[:])

            # ---- proj + residual -> x2T (fp32) ----
            x2_sb = _tl(x2p, [P, CP, OWN], F32, "x2")
            for rt in range(CP if not os.environ.get("P1_SKIP_TAIL") else 0):
                wt = _tl(wpool, [P, CP, P], BF16, "w_t")
                nc.sync.dma_start(wt[:], wproj[:, :, rt * P:(rt + 1) * P])
                ps = _tl(psb, [P, 512], F32, "ps_mm")
                for ct in range(CP):
                    nc.tensor.matmul(ps[:], wt[:, ct, :], yT_sb[:, ct, :],
                                     start=(ct == 0), stop=(ct == CP - 1))
                t = _tl(tmpp, [P, 512], F32, "t512")
                nc.scalar.activation(t[:], ps[:], ACTF.Identity, bias=bc_sb[:, rt:rt + 1])
                nc.vector.tensor_add(x2_sb[:, rt, :], t[:], xq_sb[:, rt, :])
                nc.sync.dma_start(x2T[:, rt, :], x2_sb[:, rt, :])

            # ---- LN2 -> h2 bf16 export; fp32 chunks feed router logits ----
            ps_l = _tl(psr, [E, OWN], F32, "ps_row")

            def logits_ct(ct, hct):
                nc.tensor.matmul(ps_l[:], wrout_sb[:, ct, :], hct[:],
                                 start=(ct == 0), stop=(ct == CP - 1))
                hbf = _tl(tmpp, [P, 512], BF16, "hbf", bufs=1)
                nc.vector.tensor_copy(hbf[:], hct[:])
                nc.sync.dma_start(h2T[:, ct, :], hbf[:])

            if not os.environ.get("P1_SKIP_TAIL"):
                ln_chunk(x2_sb, OWN, ln2g_sb, ln2b_sb, ones_f, None, None,
                         ct_consumer=logits_ct)
                lg = _tl(tmpp, [E, OWN], F32, "lg", bufs=1)
                nc.vector.tensor_copy(lg[:], ps_l[:])
                nc.sync.dma_start(logitsT[:], lg[:])

    nc.compile()
    return nc


# ---------------------------------------------------------------------------
# P2: expert-parallel MoE
# ---------------------------------------------------------------------------

def build_p2(use_hw_gelu=False):
    nc = bacc.Bacc(None, target_bir_lowering=False)

    ge = nc.dram_tensor("ge", (P, CP, CAP), BF16, kind="ExternalInput")
    fcw = nc.dram_tensor("fcw", (P, CP, HID), BF16, kind="ExternalInput")
    fcb = nc.dram_tensor("fcb", (P, HP), F32, kind="ExternalInput")
    pjw = nc.dram_tensor("pjw", (P, HP, C), BF16, kind="ExternalInput")
    pjb = nc.dram_tensor("pjb", (P, CP), F32, kind="ExternalInput")
    gate = nc.dram_tensor("gate", (1, CAP), F32, kind="ExternalInput")
    out = nc.dram_tensor("out", (P, CP, CAP), F32, kind="ExternalOutput")

    chunks = []
    off = 0
    while off < CAP:
        w = min(512, CAP - off)
        chunks.append((off, w))
        off += w

    with tile.TileContext(nc) as tc:
        with (
            tc.tile_pool(name="const", bufs=1) as constp,
            tc.tile_pool(name="fcw", bufs=1) as fcwp,
            tc.tile_pool(name="pjw", bufs=2) as pjwp,
            tc.tile_pool(name="ge", bufs=1) as gep,
            tc.tile_pool(name="he", bufs=2 if use_hw_gelu else 1) as hep,
            tc.tile_pool(name="tmp", bufs=2) as tmpp,
            tc.tile_pool(name="outp", bufs=3) as outp,
            tc.tile_pool(name="ps_fc", bufs=4, space="PSUM") as psfc,
            tc.tile_pool(name="ps_pj", bufs=4, space="PSUM") as pspj,
        ):
            fcw_sb = fcwp.tile([P, CP, HID], BF16)
            ge_sb = gep.tile([P, CP, CAP], BF16)
            for ct in range(CP):
                nc.sync.dma_start(ge_sb[:, ct, :], ge[:, ct, :])
                nc.sync.dma_start(fcw_sb[:, ct, :], fcw[:, ct, :])
            fcb_sb = constp.tile([P, HP], F32); nc.sync.dma_start(fcb_sb[:], fcb[:])
            pjb_sb = constp.tile([P, CP], F32); nc.sync.dma_start(pjb_sb[:], pjb[:])
            gate_row = constp.tile([1, CAP], F32); nc.sync.dma_start(gate_row[:], gate[:])
            gate128 = constp.tile([P, CAP], F32)
            nc.gpsimd.partition_broadcast(gate128[:], gate_row[:])

            for off, tw in chunks:
                tsl = slice(off, off + tw)
                he_sb = _tl(hep, [P, HP, 512], BF16, "he")
                for rt in range(HP):
                    ps = _tl(psfc, [P, 512], F32, "ps_fc", tw)
                    for ct in range(CP):
                        nc.tensor.matmul(ps[:], fcw_sb[:, ct, rt * P:(rt + 1) * P],
                                         ge_sb[:, ct, tsl],
                                         start=(ct == 0), stop=(ct == CP - 1))
                    if use_hw_gelu:
                        fn = (ACTF.Tanh if use_hw_gelu == "stub"
                              else ACTF.Gelu_apprx_tanh)
                        nc.scalar.activation(he_sb[:, rt, :tw], ps[:], fn,
                                             bias=fcb_sb[:, rt:rt + 1])
                    else:
                        xx = _tl(tmpp, [P, 512], F32, "xx", tw)
                        nc.scalar.activation(xx[:], ps[:], ACTF.Identity,
                                             bias=fcb_sb[:, rt:rt + 1])
                        x3 = _tl(tmpp, [P, 512], F32, "x3", tw)
                        nc.vector.tensor_mul(x3[:], xx[:], xx[:])
                        nc.vector.tensor_mul(x3[:], x3[:], xx[:])
                        nc.vector.scalar_tensor_tensor(
                            x3[:], x3[:], 0.044715, xx[:], ALU.mult, ALU.add)
                        uu = _tl(tmpp, [P, 512], F32, "uu", tw)
                        nc.scalar.activation(uu[:], x3[:], ACTF.Tanh, scale=GELU_C)
                        nc.vector.tensor_scalar(uu[:], uu[:], 1.0, 0.5, ALU.add, ALU.mult)
                        nc.vector.tensor_mul(he_sb[:, rt, :tw], xx[:], uu[:])
                for rt2 in range(CP):
                    pw = _tl(pjwp, [P, HP, P], BF16, "pjw_rt")
                    nc.sync.dma_start(pw[:], pjw[:, :, rt2 * P:(rt2 + 1) * P])
                    ps2 = _tl(pspj, [P, 512], F32, "ps_pj", tw)
                    for ht in range(HP):
                        nc.tensor.matmul(ps2[:], pw[:, ht, :], he_sb[:, ht, :tw],
                                         start=(ht == 0), stop=(ht == HP - 1))
                    o = _tl(outp, [P, 512], F32, "o", tw)
                    nc.scalar.activation(o[:], ps2[:], ACTF.Identity,
                                         bias=pjb_sb[:, rt2:rt2 + 1])
                    nc.vector.tensor_mul(o[:], o[:], gate128[:, tsl])
                    nc.sync.dma_start(out[:, rt2, tsl], o[:])

    nc.compile()
    return nc


# ---------------------------------------------------------------------------
# Host orchestration
# ---------------------------------------------------------------------------

_cache = {}


def _get_programs():
    if "p1" not in _cache:
        _cache["p1"] = build_p1()
    if "p2" not in _cache:
        _cache["p2"] = build_p2(use_hw_gelu=True)
    return _cache["p1"], _cache["p2"]


def _own_blocks(c):
    return [(0, c), (0, NB - 1 - c), (1, c), (1, NB - 1 - c)]


def _run_p1(p1, x, ln1_g, ln1_b, ln2_g, ln2_b, attn_w, attn_b, proj_w,
            proj_b, router_w):
    xT = np.ascontiguousarray(x.reshape(N, C).T)            # [C, N] fp32
    xT_bf = xT.astype(BF)
    xkv_h = _part3(xT_bf.astype(np.float32)).astype(BF)
    qpos_h, xq_h = [], []
    for c in range(NCORES):
        cols = []
        qp = np.zeros((P, 4), np.float32)
        for s, (b, j) in enumerate(_own_blocks(c)):
            cols.append(np.arange(b * T + j * P, b * T + (j + 1) * P))
            qp[:, s] = np.arange(j * P, (j + 1) * P, dtype=np.float32)
        cols = np.concatenate(cols)
        xq_h.append(_part3(np.ascontiguousarray(xT[:, cols])))
        qpos_h.append(qp)
    kpos_h = np.arange(T, dtype=np.float32).reshape(1, T)
    wqk_h = _part3(attn_w[:, :2 * C]).astype(BF)
    wv_h = _part3(attn_w[:, 2 * C:]).astype(BF)
    bqk_h = _part2(attn_b[:2 * C])
    wproj_h = _part3(proj_w).astype(BF)
    bc_h = _part2(proj_w.T @ attn_b[2 * C:] + proj_b)
    wrout_h = _part3(router_w)
    ln1g_h, ln1b_h = _part2(ln1_g), _part2(ln1_b)
    ln2g_h, ln2b_h = _part2(ln2_g), _part2(ln2_b)

    in_maps1 = []
    for c in range(NCORES):
        in_maps1.append({
            "xkv": xkv_h, "xq": xq_h[c], "qpos": qpos_h[c], "kpos": kpos_h,
            "wqk": wqk_h, "wv": wv_h, "bqk": bqk_h, "wproj": wproj_h,
            "bc": bc_h, "ln1g": ln1g_h, "ln1b": ln1b_h,
            "ln2g": ln2g_h, "ln2b": ln2b_h, "wrout": wrout_h,
        })
    res1 = run_bass_kernel_spmd(p1, in_maps1, core_ids=list(range(NCORES)))

    x2T_full = np.zeros((C, N), np.float32)
    h2T_full = np.zeros((C, N), BF)
    logits = np.zeros((N, E), np.float32)
    for c in range(NCORES):
        r = res1.results[c]
        x2c = r["x2T"].transpose(1, 0, 2).reshape(C, OWN)
        h2c = r["h2T"].transpose(1, 0, 2).reshape(C, OWN)
        lgc = r["logitsT"]
        for s, (b, j) in enumerate(_own_blocks(c)):
            cols = np.arange(b * T + j * P, b * T + (j + 1) * P)
            x2T_full[:, cols] = x2c[:, s * P:(s + 1) * P]
            h2T_full[:, cols] = h2c[:, s * P:(s + 1) * P]
            logits[cols] = lgc[:, s * P:(s + 1) * P].T
    return x2T_full, h2T_full, logits, xT, xT_bf


def kernel(**inputs):
    x = np.asarray(inputs["x"], np.float32)
    ln1_g = np.asarray(inputs["ln1_g"], np.float32)
    ln1_b = np.asarray(inputs["ln1_b"], np.float32)
    ln2_g = np.asarray(inputs["ln2_g"], np.float32)
    ln2_b = np.asarray(inputs["ln2_b"], np.float32)
    attn_w = np.asarray(inputs["attn_w"], np.float32)
    attn_b = np.asarray(inputs["attn_b"], np.float32)
    proj_w = np.asarray(inputs["proj_w"], np.float32)
    proj_b = np.asarray(inputs["proj_b"], np.float32)
    router_w = np.asarray(inputs["router_w"], np.float32)
    fc_w = np.asarray(inputs["fc_w"], np.float32)
    fc_b = np.asarray(inputs["fc_b"], np.float32)
    pj_w = np.asarray(inputs["pj_w"], np.float32)
    pj_b = np.asarray(inputs["pj_b"], np.float32)

    p1, p2 = _get_programs()
    x2T_full, h2T_full, logits, xT, xT_bf = _run_p1(
        p1, x, ln1_g, ln1_b, ln2_g, ln2_b, attn_w, attn_b, proj_w, proj_b,
        router_w)

    lm = logits.max(-1, keepdims=True)
    probs = np.exp(logits - lm)
    probs /= probs.sum(-1, keepdims=True)
    topk_i = np.argsort(-probs, axis=-1, kind="stable")[:, :TOPK]
    topk_p = np.take_along_axis(probs, topk_i, axis=-1)
    topk_p = topk_p / topk_p.sum(-1, keepdims=True)

    idx_e, gate_e, overflow = [], [], []
    for e in range(E):
        rows, ks = np.nonzero(topk_i == e)
        g = topk_p[rows, ks]
        if len(rows) > CAP:
            overflow.append((e, rows[CAP:], g[CAP:]))
            rows, g = rows[:CAP], g[:CAP]
        idx_e.append(rows)
        gate_e.append(g)

    in_maps2 = []
    for e in range(E):
        n_e = len(idx_e[e])
        ge = np.zeros((C, CAP), np.float32)
        ge[:, :n_e] = h2T_full[:, idx_e[e]].astype(np.float32)
        gt = np.zeros((1, CAP), np.float32)
        gt[0, :n_e] = gate_e[e]
        in_maps2.append({
            "ge": _part3(ge).astype(BF),
            "fcw": _part3(fc_w[e]).astype(BF),
            "fcb": _part2(fc_b[e]),
            "pjw": _part3(pj_w[e]).astype(BF),
            "pjb": _part2(pj_b[e]),
            "gate": gt,
        })
    res2 = run_bass_kernel_spmd(p2, in_maps2, core_ids=list(range(NCORES)))

    outT = x2T_full
    for e in range(E):
        n_e = len(idx_e[e])
        oe = res2.results[e]["out"].transpose(1, 0, 2).reshape(C, CAP)
        outT[:, idx_e[e]] += oe[:, :n_e]

    for e, rows, g in overflow:
        h2o = h2T_full[:, rows].astype(np.float32).T
        he = h2o @ fc_w[e] + fc_b[e]
        he = 0.5 * he * (1.0 + np.tanh(GELU_C * (he + 0.044715 * he ** 3)))
        oe = (he @ pj_w[e] + pj_b[e]) * g[:, None]
        outT[:, rows] += oe.T

    return np.ascontiguousarray(outT.T).reshape(B, T, C).astype(np.float32)



# revision 23
# speedup vs baseline: 2.1036x; 2.1036x over previous
"""Trainium2 Bass kernel for a transformer block with top-2-of-8 MoE FFN.

Strategy (8 NeuronCores, SPMD via run_bass_kernel_spmd, 4 uniform launches):
  A  (qkv):  token-parallel. Each core projects qkv for its own 512 tokens
             (LN1 done on host between launches). No duplicated work.
  B1 (attn): head-parallel. Each core runs causal attention for 4
             (batch, head) units (2 heads x 2 batches, all 4096 queries).
             Scores are computed transposed (k on partitions, q on free
             axis) so no on-device transposes are needed; causal block
             skipping; softmax denominator via a ones-column folded into
             the v stationary operand. Which heads a core owns is baked
             into the host-packed inputs, so the program is uniform.
  B2 (proj): token-parallel output projection (y @ proj_w + b), fp32 out.
  C  (moe):  expert-parallel. Core e runs gelu(h2 @ fc_w[e]) @ pj_w[e]
             for tokens routed to expert e (capacity CAP), gate applied
             on device.
  Host between launches: LN1, q/k/v redistribution, y assembly, residual,
  LN2, router softmax + top-2, per-expert gather, final scatter-add.
"""

import math
import numpy as np
import ml_dtypes

import concourse.bass as bass
import concourse.mybir as mybir
import concourse.tile as tile
from concourse import bacc
from concourse.bass_utils import run_bass_kernel_spmd

F32 = mybir.dt.float32
BF16 = mybir.dt.bfloat16
AX = mybir.AxisListType
ALU = mybir.AluOpType
ACTF = mybir.ActivationFunctionType

B, T, C, H, E, TOPK = 2, 2048, 1024, 16, 8, 2
HID = 4 * C
D = C // H            # 64
N = B * T             # 4096 tokens
P = 128
CP = C // P           # 8 c-tiles
HP = HID // P         # 32 hid-tiles
NB = T // P           # 16 query/key blocks per batch
NCORES = 8
OWN = 4 * P           # 512 own tokens per core
NU = 4                # attention units (batch, head) per core
CAP = 1088            # per-expert token capacity (max observed 1082)
INV_SQRT_D = 1.0 / math.sqrt(D)
GELU_C = math.sqrt(2.0 / math.pi)

BF = ml_dtypes.bfloat16

_tcnt = [0]


def _tl(pool, shape, dtype, tag, ncol=None, bufs=None):
    _tcnt[0] += 1
    kw = {} if bufs is None else {"bufs": bufs}
    t = pool.tile(shape, dtype, tag=tag, name=f"{tag}_{_tcnt[0]}", **kw)
    return t if ncol is None else t[:, :ncol]


def _part3(a, p=P):
    """[R, F] -> [p, R//p, F]; row r = o*p + pi maps to [pi, o, F]."""
    R, Fd = a.shape
    return np.ascontiguousarray(a.reshape(R // p, p, Fd).transpose(1, 0, 2))


def _part2(a, p=P):
    """[R] -> [p, R//p]; row r = o*p + pi."""
    a = np.asarray(a, np.float32)
    R = a.shape[0]
    return np.ascontiguousarray(a.reshape(R // p, p).T)


def _own_cols(c):
    cols = []
    for b, j in [(0, c), (0, NB - 1 - c), (1, c), (1, NB - 1 - c)]:
        cols.append(np.arange(b * T + j * P, b * T + (j + 1) * P))
    return np.concatenate(cols)


def _tile_w(w, dtype=None):
    """[K, M] weight -> [M//P, P, (K//P)*P] pre-tiled: one contiguous
    [P, K//P * P] stationary tile per output row-tile."""
    w3 = _part3(w)                      # [P, K//P, M]
    M = w.shape[1]
    out = np.stack([np.ascontiguousarray(
        w3[:, :, rt * P:(rt + 1) * P]).reshape(P, -1)
        for rt in range(M // P)])
    return out.astype(dtype) if dtype is not None else out


# ---------------------------------------------------------------------------
# Launch A: qkv projection for own 512 tokens (LN1 input prepared on host)
# ---------------------------------------------------------------------------

def build_qkv():
    nc = bacc.Bacc(None, target_bir_lowering=False)

    hT = nc.dram_tensor("hT", (P, CP, OWN), BF16, kind="ExternalInput")
    # weights pre-tiled on host: wqkv[rt] is one contiguous [P, CP*P] tile
    wqkv = nc.dram_tensor("wqkv", (3 * CP, P, CP * P), BF16, kind="ExternalInput")
    bqkv = nc.dram_tensor("bqkv", (P, 3 * CP), F32, kind="ExternalInput")
    qkvT = nc.dram_tensor("qkvT", (P, 3 * CP, OWN), BF16, kind="ExternalOutput")

    with tile.TileContext(nc) as tc:
        with (
            tc.tile_pool(name="const", bufs=1) as constp,
            tc.tile_pool(name="wpool", bufs=3) as wpool,
            tc.tile_pool(name="out", bufs=3) as outp,
            tc.tile_pool(name="ps_mm", bufs=4, space="PSUM") as psb,
        ):
            bqkv_sb = constp.tile([P, 3 * CP], F32)
            nc.sync.dma_start(bqkv_sb[:], bqkv[:])
            h_sb = constp.tile([P, CP, OWN], BF16)
            nc.sync.dma_start(h_sb[:], hT[:])

            for rt in range(3 * CP):
                wt = _tl(wpool, [P, CP * P], BF16, "w_t")
                nc.sync.dma_start(wt[:], wqkv[rt])
                ps = _tl(psb, [P, OWN], F32, "ps_mm")
                for ct in range(CP):
                    nc.tensor.matmul(ps[:], wt[:, ct * P:(ct + 1) * P],
                                     h_sb[:, ct, :],
                                     start=(ct == 0), stop=(ct == CP - 1))
                o = _tl(outp, [P, OWN], BF16, "o")
                nc.scalar.activation(o[:], ps[:], ACTF.Identity,
                                     bias=bqkv_sb[:, rt:rt + 1])
                nc.sync.dma_start(qkvT[:, rt, :], o[:])

    nc.compile()
    return nc


# ---------------------------------------------------------------------------
# Launch B1: head-parallel causal attention (4 units per core)
# ---------------------------------------------------------------------------

def build_attn():
    nc = bacc.Bacc(None, target_bir_lowering=False)

    # units u=0..3 packed: partitions (u%2)*64.., index u//2
    qTu = nc.dram_tensor("qTu", (P, 2, T), BF16, kind="ExternalInput")
    kTu = nc.dram_tensor("kTu", (P, 2, T), BF16, kind="ExternalInput")
    vau = nc.dram_tensor("vau", (P, NU * NB, D + 1), BF16, kind="ExternalInput")
    tri = nc.dram_tensor("tri", (P, P), BF16, kind="ExternalInput")
    # unnormalized y (rows 0..D-1) + softmax denominator (row D); host divides
    yTu = nc.dram_tensor("yTu", (D + 1, NU, T), BF16, kind="ExternalOutput")

    with tile.TileContext(nc) as tc:
        with (
            tc.tile_pool(name="const", bufs=1) as constp,
            tc.tile_pool(name="pexp", bufs=2) as pexpp,
            tc.tile_pool(name="y", bufs=1) as yp,
            tc.tile_pool(name="ps_s", bufs=3, space="PSUM") as pss,
            tc.tile_pool(name="ps_y", bufs=2, space="PSUM") as psy,
        ):
            tri_sb = constp.tile([P, P], BF16)
            nc.sync.dma_start(tri_sb[:], tri[:])
            q_sb = constp.tile([P, 2, T], BF16)
            nc.sync.dma_start(q_sb[:], qTu[:])
            k_sb = constp.tile([P, 2, T], BF16)
            nc.sync.dma_start(k_sb[:], kTu[:])
            va_sb = constp.tile([P, NU * NB, D + 1], BF16)
            for s in range(NU):
                nc.sync.dma_start(va_sb[:, s * NB:(s + 1) * NB, :],
                                  vau[:, s * NB:(s + 1) * NB, :])

            y_sb = yp.tile([D + 1, NU, T], BF16)

            # chunk kc occupies cols off[kc] .. off[kc] + (NB-kc)*P in pex_all
            off = [0] * (NB + 1)
            for kc in range(NB):
                off[kc + 1] = off[kc] + (NB - kc) * P

            for u in range(NU):
                po = (u % 2) * D
                u2 = u // 2
                # phase 1: scores + exp for all chunks, stored to SBUF
                pex = _tl(pexpp, [P, off[NB]], BF16, "pexp")
                for kc in range(NB):
                    w = (NB - kc) * P
                    for g in range((w + 511) // 512):
                        wg = min(512, w - 512 * g)
                        ps_sc = _tl(pss, [P, 512], F32, "ps_s", wg)
                        nc.tensor.matmul(
                            ps_sc[:],
                            k_sb[po:po + D, u2, kc * P:(kc + 1) * P],
                            q_sb[po:po + D, u2,
                                 kc * P + 512 * g:kc * P + 512 * g + wg],
                            start=True, stop=True)
                        nc.scalar.activation(
                            pex[:, off[kc] + 512 * g:off[kc] + 512 * g + wg],
                            ps_sc[:], ACTF.Exp, scale=INV_SQRT_D)
                    # mask the diagonal block (query block j == kc)
                    nc.gpsimd.tensor_mul(pex[:, off[kc]:off[kc] + P],
                                         pex[:, off[kc]:off[kc] + P], tri_sb[:])
                # phase 2: one AV accumulation chain per query block
                for j in range(NB):
                    ps_yd = _tl(psy, [D + 1, P], F32, "ps_y")
                    for kc in range(j + 1):
                        nc.tensor.matmul(
                            ps_yd[:], va_sb[:, u * NB + kc, :],
                            pex[:, off[kc] + (j - kc) * P:
                                 off[kc] + (j - kc + 1) * P],
                            start=(kc == 0), stop=(kc == j))
                    nc.vector.tensor_copy(y_sb[:, u, j * P:(j + 1) * P],
                                          ps_yd[:])

            for u in range(NU):
                nc.sync.dma_start(yTu[:, u, :], y_sb[:, u, :])

    nc.compile()
    return nc


# ---------------------------------------------------------------------------
# Launch B2: output projection for own 512 tokens
# ---------------------------------------------------------------------------

def build_proj():
    nc = bacc.Bacc(None, target_bir_lowering=False)

    yT = nc.dram_tensor("yT", (P, CP, OWN), BF16, kind="ExternalInput")
    wproj = nc.dram_tensor("wproj", (CP, P, CP * P), BF16, kind="ExternalInput")
    bc = nc.dram_tensor("bc", (P, CP), F32, kind="ExternalInput")
    poT = nc.dram_tensor("poT", (P, CP, OWN), BF16, kind="ExternalOutput")

    with tile.TileContext(nc) as tc:
        with (
            tc.tile_pool(name="const", bufs=1) as constp,
            tc.tile_pool(name="wpool", bufs=3) as wpool,
            tc.tile_pool(name="out", bufs=3) as outp,
            tc.tile_pool(name="ps_mm", bufs=4, space="PSUM") as psb,
        ):
            bc_sb = constp.tile([P, CP], F32)
            nc.sync.dma_start(bc_sb[:], bc[:])
            y_sb = constp.tile([P, CP, OWN], BF16)
            nc.sync.dma_start(y_sb[:], yT[:])

            for rt in range(CP):
                wt = _tl(wpool, [P, CP * P], BF16, "w_t")
                nc.sync.dma_start(wt[:], wproj[rt])
                ps = _tl(psb, [P, OWN], F32, "ps_mm")
                for ct in range(CP):
                    nc.tensor.matmul(ps[:], wt[:, ct * P:(ct + 1) * P],
                                     y_sb[:, ct, :],
                                     start=(ct == 0), stop=(ct == CP - 1))
                o = _tl(outp, [P, OWN], BF16, "o")
                nc.scalar.activation(o[:], ps[:], ACTF.Identity,
                                     bias=bc_sb[:, rt:rt + 1])
                nc.sync.dma_start(poT[:, rt, :], o[:])

    nc.compile()
    return nc


# ---------------------------------------------------------------------------
# Launch C: expert-parallel MoE
# ---------------------------------------------------------------------------

def build_moe(use_hw_gelu=True):
    nc = bacc.Bacc(None, target_bir_lowering=False)

    ge = nc.dram_tensor("ge", (P, CP, CAP), BF16, kind="ExternalInput")
    fcw = nc.dram_tensor("fcw", (P, CP, HID), BF16, kind="ExternalInput")
    fcb = nc.dram_tensor("fcb", (P, HP), F32, kind="ExternalInput")
    pjw = nc.dram_tensor("pjw", (CP, P, HP * P), BF16, kind="ExternalInput")
    pjb = nc.dram_tensor("pjb", (P, CP), F32, kind="ExternalInput")
    gate = nc.dram_tensor("gate", (1, CAP), F32, kind="ExternalInput")
    out = nc.dram_tensor("out", (P, CP, CAP), BF16, kind="ExternalOutput")

    chunks = []
    off = 0
    while off < CAP:
        w = min(512, CAP - off)
        chunks.append((off, w))
        off += w

    with tile.TileContext(nc) as tc:
        with (
            tc.tile_pool(name="const", bufs=1) as constp,
            tc.tile_pool(name="fcw", bufs=1) as fcwp,
            tc.tile_pool(name="pjw", bufs=2) as pjwp,
            tc.tile_pool(name="ge", bufs=1) as gep,
            tc.tile_pool(name="he", bufs=1) as hep,
            tc.tile_pool(name="outp", bufs=3) as outp,
            tc.tile_pool(name="ps_fc", bufs=4, space="PSUM") as psfc,
            tc.tile_pool(name="ps_pj", bufs=4, space="PSUM") as pspj,
        ):
            fcw_sb = fcwp.tile([P, CP, HID], BF16)
            ge_sb = gep.tile([P, CP, CAP], BF16)
            for ct in range(CP):
                nc.sync.dma_start(ge_sb[:, ct, :], ge[:, ct, :])
                nc.sync.dma_start(fcw_sb[:, ct, :], fcw[:, ct, :])
            fcb_sb = constp.tile([P, HP], F32); nc.sync.dma_start(fcb_sb[:], fcb[:])
            pjb_sb = constp.tile([P, CP], F32); nc.sync.dma_start(pjb_sb[:], pjb[:])
            gate_row = constp.tile([1, CAP], F32); nc.sync.dma_start(gate_row[:], gate[:])
            gate128 = constp.tile([P, CAP], F32)
            nc.gpsimd.partition_broadcast(gate128[:], gate_row[:])

            # fc stage: he for all chunks, kept resident
            he_sb = hep.tile([P, HP, CAP], BF16)
            fn = ACTF.Gelu_apprx_tanh if use_hw_gelu is True else ACTF.Tanh
            for off, tw in chunks:
                tsl = slice(off, off + tw)
                for rt in range(HP):
                    ps = _tl(psfc, [P, 512], F32, "ps_fc", tw)
                    for ct in range(CP):
                        nc.tensor.matmul(ps[:], fcw_sb[:, ct, rt * P:(rt + 1) * P],
                                         ge_sb[:, ct, tsl],
                                         start=(ct == 0), stop=(ct == CP - 1))
                    nc.scalar.activation(he_sb[:, rt, tsl], ps[:], fn,
                                         bias=fcb_sb[:, rt:rt + 1])
            # pj stage: each weight tile loaded once
            for rt2 in range(CP):
                pw = _tl(pjwp, [P, HP * P], BF16, "pjw_rt")
                nc.sync.dma_start(pw[:], pjw[rt2])
                for off, tw in chunks:
                    tsl = slice(off, off + tw)
                    ps2 = _tl(pspj, [P, 512], F32, "ps_pj", tw)
                    for ht in range(HP):
                        nc.tensor.matmul(ps2[:], pw[:, ht * P:(ht + 1) * P],
                                         he_sb[:, ht, tsl],
                                         start=(ht == 0), stop=(ht == HP - 1))
                    o = _tl(outp, [P, 512], BF16, "o", tw)
                    nc.scalar.activation(o[:], ps2[:], ACTF.Identity,
                                         bias=pjb_sb[:, rt2:rt2 + 1])
                    nc.vector.tensor_mul(o[:], o[:], gate128[:, tsl])
                    nc.sync.dma_start(out[:, rt2, tsl], o[:])

    nc.compile()
    return nc


# ---------------------------------------------------------------------------
# Host orchestration
# ---------------------------------------------------------------------------

_cache = {}


def _get_programs():
    if "qkv" not in _cache:
        _cache["qkv"] = build_qkv()
    if "attn" not in _cache:
        _cache["attn"] = build_attn()
    if "proj" not in _cache:
        _cache["proj"] = build_proj()
    if "moe" not in _cache:
        _cache["moe"] = build_moe(use_hw_gelu=True)
    return _cache["qkv"], _cache["attn"], _cache["proj"], _cache["moe"]


def _layernorm(x, g, b, eps=1e-5):
    mu = x.mean(-1, keepdims=True)
    var = x.var(-1, keepdims=True)
    return (x - mu) / np.sqrt(var + eps) * g + b


def _units(c):
    """(batch, head) units owned by core c."""
    return [(0, 2 * c), (0, 2 * c + 1), (1, 2 * c), (1, 2 * c + 1)]


def kernel(**inputs):
    x = np.asarray(inputs["x"], np.float32)
    ln1_g = np.asarray(inputs["ln1_g"], np.float32)
    ln1_b = np.asarray(inputs["ln1_b"], np.float32)
    ln2_g = np.asarray(inputs["ln2_g"], np.float32)
    ln2_b = np.asarray(inputs["ln2_b"], np.float32)
    attn_w = np.asarray(inputs["attn_w"], np.float32)
    attn_b = np.asarray(inputs["attn_b"], np.float32)
    proj_w = np.asarray(inputs["proj_w"], np.float32)
    proj_b = np.asarray(inputs["proj_b"], np.float32)
    router_w = np.asarray(inputs["router_w"], np.float32)
    fc_w = np.asarray(inputs["fc_w"], np.float32)
    fc_b = np.asarray(inputs["fc_b"], np.float32)
    pj_w = np.asarray(inputs["pj_w"], np.float32)
    pj_b = np.asarray(inputs["pj_b"], np.float32)

    p_qkv, p_attn, p_proj, p_moe = _get_programs()

    # ---- host: LN1 ----
    h1 = _layernorm(x, ln1_g, ln1_b).reshape(N, C)
    h1T = np.ascontiguousarray(h1.T).astype(BF)             # [C, N]

    # ---- launch A: qkv ----
    wqkv_h = _tile_w(attn_w, BF)
    bqkv_h = _part2(attn_b)
    in_mapsA = [{
        "hT": _part3(np.ascontiguousarray(h1T[:, _own_cols(c)])),
        "wqkv": wqkv_h, "bqkv": bqkv_h,
    } for c in range(NCORES)]
    resA = run_bass_kernel_spmd(p_qkv, in_mapsA, core_ids=list(range(NCORES)))

    qkvT_full = np.zeros((3 * C, N), BF)
    for c in range(NCORES):
        r = resA.results[c]["qkvT"].transpose(1, 0, 2).reshape(3 * C, OWN)
        qkvT_full[:, _own_cols(c)] = r

    # ---- launch B1: attention (head-parallel) ----
    tri_h = (np.arange(P)[None, :] >= np.arange(P)[:, None]).astype(BF)
    in_mapsB1 = []
    for c in range(NCORES):
        qTu = np.zeros((P, 2, T), BF)
        kTu = np.zeros((P, 2, T), BF)
        vau = np.zeros((P, NU * NB, D + 1), BF)
        for u, (b, h) in enumerate(_units(c)):
            po = (u % 2) * D
            u2 = u // 2
            tsl = slice(b * T, (b + 1) * T)
            qTu[po:po + D, u2, :] = qkvT_full[h * D:(h + 1) * D, tsl]
            kTu[po:po + D, u2, :] = qkvT_full[C + h * D:C + (h + 1) * D, tsl]
            vh = qkvT_full[2 * C + h * D:2 * C + (h + 1) * D, tsl]  # [D, T]
            vau[:, u * NB:(u + 1) * NB, :D] = (
                vh.T.reshape(NB, P, D).transpose(1, 0, 2))
            vau[:, u * NB:(u + 1) * NB, D] = 1.0
        in_mapsB1.append({"qTu": qTu, "kTu": kTu, "vau": vau, "tri": tri_h})
    resB1 = run_bass_kernel_spmd(p_attn, in_mapsB1, core_ids=list(range(NCORES)))

    yT_full = np.zeros((C, N), BF)                          # [C, N] head-major
    for c in range(NCORES):
        r = resB1.results[c]["yTu"].astype(np.float32)      # [D+1, NU, T]
        for u, (b, h) in enumerate(_units(c)):
            yT_full[h * D:(h + 1) * D, b * T:(b + 1) * T] = (
                r[:D, u, :] / r[D:D + 1, u, :])

    # ---- launch B2: proj ----
    wproj_h = _tile_w(proj_w, BF)
    bc_h = _part2(proj_b)
    in_mapsB2 = [{
        "yT": _part3(np.ascontiguousarray(yT_full[:, _own_cols(c)])),
        "wproj": wproj_h, "bc": bc_h,
    } for c in range(NCORES)]
    resB2 = run_bass_kernel_spmd(p_proj, in_mapsB2, core_ids=list(range(NCORES)))

    poT_full = np.zeros((C, N), np.float32)
    for c in range(NCORES):
        r = resB2.results[c]["poT"].transpose(1, 0, 2).reshape(C, OWN)
        poT_full[:, _own_cols(c)] = r.astype(np.float32)

    # ---- host: residual + LN2 + routing ----
    x2 = x.reshape(N, C) + poT_full.T                       # [N, C] fp32
    h2 = _layernorm(x2, ln2_g, ln2_b)
    logits = h2 @ router_w
    h2T_full = np.ascontiguousarray(h2.T).astype(BF)

    lm = logits.max(-1, keepdims=True)
    probs = np.exp(logits - lm)
    probs /= probs.sum(-1, keepdims=True)
    topk_i = np.argsort(-probs, axis=-1, kind="stable")[:, :TOPK]
    topk_p = np.take_along_axis(probs, topk_i, axis=-1)
    topk_p = topk_p / topk_p.sum(-1, keepdims=True)

    idx_e, gate_e, overflow = [], [], []
    for e in range(E):
        rows, ks = np.nonzero(topk_i == e)
        g = topk_p[rows, ks]
        if len(rows) > CAP:
            overflow.append((e, rows[CAP:], g[CAP:]))
            rows, g = rows[:CAP], g[:CAP]
        idx_e.append(rows)
        gate_e.append(g)

    # ---- launch C: MoE ----
    in_mapsC = []
    for e in range(E):
        n_e = len(idx_e[e])
        gecols = np.zeros((C, CAP), np.float32)
        gecols[:, :n_e] = h2T_full[:, idx_e[e]].astype(np.float32)
        gt = np.zeros((1, CAP), np.float32)
        gt[0, :n_e] = gate_e[e]
        in_mapsC.append({
            "ge": _part3(gecols).astype(BF),
            "fcw": _part3(fc_w[e]).astype(BF),
            "fcb": _part2(fc_b[e]),
            "pjw": _tile_w(pj_w[e], BF),
            "pjb": _part2(pj_b[e]),
            "gate": gt,
        })
    resC = run_bass_kernel_spmd(p_moe, in_mapsC, core_ids=list(range(NCORES)))

    out = x2                                                # [N, C] fp32
    for e in range(E):
        n_e = len(idx_e[e])
        oe = resC.results[e]["out"].transpose(1, 0, 2).reshape(C, CAP)
        out[idx_e[e]] += oe[:, :n_e].T.astype(np.float32)

    for e, rows, g in overflow:
        h2o = h2T_full[:, rows].astype(np.float32).T
        he = h2o @ fc_w[e] + fc_b[e]
        he = 0.5 * he * (1.0 + np.tanh(GELU_C * (he + 0.044715 * he ** 3)))
        oe = (he @ pj_w[e] + pj_b[e]) * g[:, None]
        out[rows] += oe

    return np.ascontiguousarray(out).reshape(B, T, C).astype(np.float32)
